# revision 1
# baseline (speedup 1.0000x reference)
"""GeneralCRF pseudo-likelihood loss on 8 NeuronCores.

Sharding: pure data parallel over batch axis B=8 -> one batch element per
core (per spec sharding_hint). Small params (bin_embed, W_dyn, b_dyn) are
replicated. Each core computes its batch's pll and mask count; host combines
the 8 scalar pairs into the final nll (the "gather/unshard" step).
"""
import os
import numpy as np

os.environ.setdefault("JAX_PLATFORMS", "axon,cpu")

import jax
import jax.numpy as jnp

try:
    jax.config.update("jax_compilation_cache_dir", "/tmp/jax_cc_cache")
    jax.config.update("jax_persistent_cache_min_entry_size_bytes", -1)
    jax.config.update("jax_persistent_cache_min_compile_time_secs", 0.0)
except Exception:
    pass

BEAM = 64
_N_CORES = 8


def _per_batch(unaries, masks, binary_edges, binary_masks, node_features,
               targets, bin_embed, W_dyn, b_dyn):
    N, S = unaries.shape
    D = bin_embed.shape[-1]
    mf = masks.astype(unaries.dtype)
    un = unaries * mf[:, None]

    big = jnp.asarray(1e30, un.dtype)
    _un = un.at[jnp.arange(N), targets].set(big)
    _, beam_targets = jax.lax.top_k(_un, BEAM)              # [N,K]
    beam_unary = jnp.take_along_axis(un, beam_targets, axis=1)

    e0, e1 = binary_edges[:, 0], binary_edges[:, 1]         # [E]
    bt0 = beam_targets[e0]                                  # [E,K]
    bt1 = beam_targets[e1]
    s0 = bin_embed[0][bt0]                                  # [E,K,D]
    s1 = bin_embed[1][bt1]
    f0 = node_features[e0]                                  # [E,F]
    f1 = node_features[e1]
    feat = jnp.concatenate([f0, f1], axis=-1)               # [E,2F]
    ew = jax.nn.relu(feat @ W_dyn + b_dyn)                  # [E,D*D]
    ew = ew.reshape(-1, D, D)

    bin_phis = jnp.einsum('eki,eij,elj->ekl', s0, ew, s1)   # [E,K,K]

    norm_unary = jax.nn.log_softmax(beam_unary, axis=-1)
    gold_unary = jnp.where(masks, norm_unary[:, 0], 0.0)
    pll = gold_unary.sum()

    E = bin_phis.shape[0]
    norm_bin = jax.nn.log_softmax(bin_phis.reshape(E, BEAM * BEAM), axis=-1)
    gold_bin = jnp.where(binary_masks, norm_bin[:, 0], 0.0)
    pll = pll + gold_bin.sum()
    return jnp.stack([pll, mf.sum()])


_jitted = {}


def _get_fn(dev):
    if dev not in _jitted:
        _jitted[dev] = jax.jit(_per_batch, device=dev)
    return _jitted[dev]


def kernel(unaries, masks, binary_edges, binary_masks, node_features,
           targets, bin_embed, W_dyn, b_dyn):
    devs = [d for d in jax.devices() if d.platform != "cpu"][:_N_CORES]
    if not devs:
        devs = jax.devices()[:1]
    B = unaries.shape[0]

    edges32 = np.asarray(binary_edges, np.int32)
    targets32 = np.asarray(targets, np.int32)
    bin_embed_f = np.asarray(bin_embed, np.float32)
    W_f = np.asarray(W_dyn, np.float32)
    b_f = np.asarray(b_dyn, np.float32)

    outs = []
    for b in range(B):
        dev = devs[b % len(devs)]
        fn = _get_fn(dev)
        args = [jax.device_put(x, dev) for x in (
            np.asarray(unaries[b], np.float32), np.asarray(masks[b]),
            edges32[b], np.asarray(binary_masks[b]),
            np.asarray(node_features[b], np.float32), targets32[b],
            bin_embed_f, W_f, b_f)]
        outs.append(fn(*args))

    res = np.stack([np.asarray(o) for o in outs])           # [B,2]
    pll, msum = res[:, 0], res[:, 1]
    nll = -np.mean(pll / msum)
    return np.asarray(nll, np.float32)
